# revision 2
# baseline (speedup 1.0000x reference)
"""Trainium2 Bass kernel for GrowingFieldV2 GNN message passing.

Data-parallel over batch: 8 NeuronCores, each processing a 1024-row shard
of x. Small [500,*] parameters (positions/features/weights) are replicated
and the [500,500] connectivity matrix is computed redundantly on every core.

Per-core device program:
  phase 0: build conn_effT = (I + 0.5*conn_w).T from positions/features
  phase 1: actT = (x @ iw.T).T * input_gate + bias     (bf16 matmuls)
  phase 2: 3x message passing  actT = clamp(conn_effT.T @ actT, 0, 50)
  phase 3: yT = (ow * out_gate).T-contracted output    -> [10, 1024]

Host only reshapes/transposes inputs for DMA-friendly layout and
concatenates the 8 output shards.
"""

import sys

for _p in ("/opt/trn_rl_repo",):
    if _p not in sys.path:
        sys.path.insert(0, _p)

import numpy as np

N = 500            # neurons
IN = 3072          # input size
FD = 64            # feature dim
OUT = 10           # output size
B = 8192           # full batch
NCORES = 8
BS = B // NCORES   # 1024 per-core batch shard
RADIUS = 20.0
VOL = 100.0
N_ITER = 3

NT = 4             # neuron tiles
NP = N // NT       # 125 neurons per tile
KT = IN // 128     # 24 contraction tiles for phase 1
NCH = 2            # batch chunks of 512 (PSUM bank width)
CH = BS // NCH     # 512

_CACHE = {}


def _build():
    import concourse.bacc as bacc
    import concourse.tile as tile
    import concourse.bass as bass
    import concourse.mybir as mybir

    f32 = mybir.dt.float32
    bf16 = mybir.dt.bfloat16
    AF = mybir.ActivationFunctionType
    ALU = mybir.AluOpType
    PSUM = bass.MemorySpace.PSUM

    nc = bacc.Bacc("TRN2", target_bir_lowering=False, debug=False,
                   num_devices=NCORES)

    xT_d = nc.dram_tensor("xT", [IN, BS], bf16, kind="ExternalInput").ap()
    iwT_d = nc.dram_tensor("iwT", [IN, N], bf16, kind="ExternalInput").ap()
    pos_d = nc.dram_tensor("pos", [N, 3], f32, kind="ExternalInput").ap()
    posT_d = nc.dram_tensor("posT", [3, N], f32, kind="ExternalInput").ap()
    featT_d = nc.dram_tensor("featT", [FD, N], f32, kind="ExternalInput").ap()
    ow_d = nc.dram_tensor("ow", [N, OUT], f32, kind="ExternalInput").ap()
    bias_d = nc.dram_tensor("bias", [N, 1], f32, kind="ExternalInput").ap()
    yT_d = nc.dram_tensor("yT", [OUT, BS], f32, kind="ExternalOutput").ap()

    import os
    WARMUP = int(os.environ.get("BASS_WARMUP", "0"))

    with tile.TileContext(nc) as tc:
        with (
            tc.tile_pool(name="wts", bufs=1) as wts,
            tc.tile_pool(name="xstage", bufs=3) as xstage,
            tc.tile_pool(name="xbfp", bufs=3) as xbfp,
            tc.tile_pool(name="acts", bufs=2) as acts,
            tc.tile_pool(name="cwork", bufs=2) as cwork,
            tc.tile_pool(name="small", bufs=1) as small,
            tc.tile_pool(name="ps", bufs=1, space=PSUM) as ps,
        ):
            # ---------- tensor-engine warmup (starts the DVFS ramp early) ----
            if WARMUP:
                wu_lhs = small.tile([128, 128], bf16, tag="wulhs")
                nc.vector.memset(wu_lhs[:], 0.0)
                wu_rhs = small.tile([128, 512], bf16, tag="wurhs")
                nc.vector.memset(wu_rhs[:], 0.0)
                wu_ps = ps.tile([128, 512], f32, tag="ps0", name="wu_ps")
                for _w in range(WARMUP):
                    nc.tensor.matmul(wu_ps[:, 0:128], wu_lhs[:],
                                     wu_rhs[:, 0:128], start=True, stop=True)

            # ---------- small parameter DMAs ----------
            posT_sb = small.tile([3, N], f32, tag="posT")
            nc.sync.dma_start(out=posT_sb[:], in_=posT_d[:])
            featT_sb = small.tile([FD, N], f32, tag="featT")
            nc.sync.dma_start(out=featT_sb[:], in_=featT_d[:])

            pos_m = []
            ow_m = []
            bias_m = []
            for m in range(NT):
                pt = small.tile([NP, 3], f32, tag=f"pos{m}")
                nc.sync.dma_start(out=pt[:], in_=pos_d[m * NP:(m + 1) * NP, :])
                pos_m.append(pt)
                ot = small.tile([NP, OUT], f32, tag=f"ow{m}")
                nc.sync.dma_start(out=ot[:], in_=ow_d[m * NP:(m + 1) * NP, :])
                ow_m.append(ot)
                bt = small.tile([NP, 1], f32, tag=f"bias{m}")
                nc.sync.dma_start(out=bt[:], in_=bias_d[m * NP:(m + 1) * NP, :])
                bias_m.append(bt)

            # clip positions into the volume (per reference)
            posTc = small.tile([3, N], f32, tag="posTc")
            nc.vector.tensor_scalar(out=posTc[:], in0=posT_sb[:],
                                    scalar1=0.1, scalar2=VOL - 0.1,
                                    op0=ALU.max, op1=ALU.min)
            # centered copy for the Gram-based pairwise distances
            posTcc = small.tile([3, N], f32, tag="posTcc")
            nc.vector.tensor_scalar(out=posTcc[:], in0=posTc[:],
                                    scalar1=50.0, scalar2=None,
                                    op0=ALU.subtract)
            pos2 = small.tile([3, N], f32, tag="pos2")
            nc.vector.tensor_mul(pos2[:], posTcc[:], posTcc[:])
            feat2 = small.tile([FD, N], f32, tag="feat2")
            nc.vector.tensor_mul(feat2[:], featT_sb[:], featT_sb[:])

            posx_m = []   # clipped x-coordinate columns [125,1]
            for m in range(NT):
                pc = small.tile([NP, 1], f32, tag=f"posx{m}")
                nc.vector.tensor_scalar(out=pc[:], in0=pos_m[m][:, 0:1],
                                        scalar1=0.1, scalar2=VOL - 0.1,
                                        op0=ALU.max, op1=ALU.min)
                posx_m.append(pc)

            ones3 = small.tile([3, 1], f32, tag="ones3")
            nc.vector.memset(ones3[:], 1.0)
            ones64 = small.tile([FD, 1], f32, tag="ones64")
            nc.vector.memset(ones64[:], 1.0)
            ones1 = small.tile([1, NP], f32, tag="ones1")
            nc.vector.memset(ones1[:], 1.0)

            # ---------- tiny PE matmuls (borrow phase-1 psum tags) ----------
            # r2 row: sum over 3 coords of centered pos^2 -> [1, N]
            r2_ps = ps.tile([1, N], f32, tag="ps0")
            nc.tensor.matmul(r2_ps[:], ones3[:], pos2[:], start=True, stop=True)
            r2row = small.tile([1, N], f32, tag="r2row")
            nc.vector.tensor_copy(r2row[:], r2_ps[:])

            # feature norm row: sqrt(sum f^2) -> max eps -> reciprocal
            f2_ps = ps.tile([1, N], f32, tag="ps1")
            nc.tensor.matmul(f2_ps[:], ones64[:], feat2[:], start=True, stop=True)
            nrm = small.tile([1, N], f32, tag="nrm")
            nc.scalar.activation(nrm[:], f2_ps[:], AF.Sqrt)
            nrm2 = small.tile([1, N], f32, tag="nrm2")
            nc.vector.tensor_scalar(out=nrm2[:], in0=nrm[:], scalar1=1e-6,
                                    scalar2=None, op0=ALU.max)
            rnrow = small.tile([1, N], f32, tag="rnrow")
            nc.vector.reciprocal(rnrow[:], nrm2[:])

            # gating rows (use clipped, uncentered x coords)
            igrow = small.tile([1, N], f32, tag="igrow")
            nc.scalar.activation(igrow[:], posTc[0:1, :], AF.Exp, scale=-2.0 / VOL)
            igsum = small.tile([1, 1], f32, tag="igsum")
            nc.vector.reduce_sum(igsum[:], igrow[:], axis=mybir.AxisListType.X)
            igs_ps = ps.tile([NP, 1], f32, tag="ps2", name="igs_ps")
            nc.tensor.matmul(igs_ps[:], ones1[:], igsum[:], start=True, stop=True)
            igsum2 = small.tile([NP, 1], f32, tag="igsum2")
            nc.vector.tensor_scalar(out=igsum2[:], in0=igs_ps[:], scalar1=1e-6,
                                    scalar2=None, op0=ALU.add)
            igb = small.tile([NP, 1], f32, tag="igb")
            nc.vector.reciprocal(igb[:], igsum2[:])

            neg2_row = small.tile([1, 1], f32, tag="neg2row")
            nc.vector.memset(neg2_row[:], -2.0)
            neg2_col = small.tile([NP, 1], f32, tag="neg2col")
            nc.vector.memset(neg2_col[:], -2.0)

            ogrow = small.tile([1, N], f32, tag="ogrow")
            nc.scalar.activation(ogrow[:], posTc[0:1, :], AF.Exp,
                                 scale=2.0 / VOL, bias=neg2_row[:])
            ogsum = small.tile([1, 1], f32, tag="ogsum")
            nc.vector.reduce_sum(ogsum[:], ogrow[:], axis=mybir.AxisListType.X)
            ogs_ps = ps.tile([NP, 1], f32, tag="ps3", name="ogs_ps")
            nc.tensor.matmul(ogs_ps[:], ones1[:], ogsum[:], start=True, stop=True)
            ogsum2 = small.tile([NP, 1], f32, tag="ogsum2")
            nc.vector.tensor_scalar(out=ogsum2[:], in0=ogs_ps[:], scalar1=1e-6,
                                    scalar2=None, op0=ALU.add)
            ogb = small.tile([NP, 1], f32, tag="ogb")
            nc.vector.reciprocal(ogb[:], ogsum2[:])

            # broadcast rows to [125, N] tiles via PE ones-matmul
            r2b_ps = ps.tile([NP, N], f32, tag="ps0", name="r2b_ps")
            nc.tensor.matmul(r2b_ps[:], ones1[:], r2row[:], start=True, stop=True)
            r2b = small.tile([NP, N], f32, tag="r2b")
            nc.vector.tensor_copy(r2b[:], r2b_ps[:])
            rnb_ps = ps.tile([NP, N], f32, tag="ps1", name="rnb_ps")
            nc.tensor.matmul(rnb_ps[:], ones1[:], rnrow[:], start=True, stop=True)
            rnb = small.tile([NP, N], f32, tag="rnb")
            nc.vector.tensor_copy(rnb[:], rnb_ps[:])

            # row -> column slices [125,1] via small DMAs
            rn_col = []
            r2_col = []
            for m in range(NT):
                rc = small.tile([NP, 1], f32, tag=f"rncol{m}")
                nc.sync.dma_start(out=rc[:], in_=rnrow[0:1, m * NP:(m + 1) * NP])
                rn_col.append(rc)
                r2c = small.tile([NP, 1], f32, tag=f"r2col{m}")
                nc.sync.dma_start(out=r2c[:], in_=r2row[0:1, m * NP:(m + 1) * NP])
                r2_col.append(r2c)

            # per-tile gate columns
            gate_m = []
            wtil_m = []
            for m in range(NT):
                ie = small.tile([NP, 1], f32, tag=f"igexp{m}")
                nc.scalar.activation(ie[:], posx_m[m][:], AF.Exp, scale=-2.0 / VOL)
                g = small.tile([NP, 1], f32, tag=f"gate{m}")
                nc.vector.tensor_mul(g[:], ie[:], igb[:])
                gate_m.append(g)

                oe = small.tile([NP, 1], f32, tag=f"ogexp{m}")
                nc.scalar.activation(oe[:], posx_m[m][:], AF.Exp,
                                     scale=2.0 / VOL, bias=neg2_col[:])
                og = small.tile([NP, 1], f32, tag=f"og{m}")
                nc.vector.tensor_mul(og[:], oe[:], ogb[:])
                w1 = small.tile([NP, OUT], f32, tag=f"wtf{m}")
                nc.vector.tensor_scalar(out=w1[:], in0=ow_m[m][:],
                                        scalar1=og[:], scalar2=None,
                                        op0=ALU.mult)
                wb = small.tile([NP, OUT], bf16, tag=f"wtb{m}")
                nc.vector.tensor_copy(wb[:], w1[:])
                wtil_m.append(wb)

            # ---------- connectivity tiles ----------
            sym_m = []
            rs_col = []
            for m in range(NT):
                gf_ps = ps.tile([NP, N], f32, tag=f"ps{m}")
                nc.tensor.matmul(gf_ps[:], featT_sb[:, m * NP:(m + 1) * NP],
                                 featT_sb[:], start=True, stop=True)
                gf_sb = cwork.tile([NP, N], f32, tag="gf")
                nc.vector.tensor_copy(gf_sb[:], gf_ps[:])

                g_ps = ps.tile([NP, N], f32, tag=f"ps{m}")
                nc.tensor.matmul(g_ps[:], posTcc[:, m * NP:(m + 1) * NP],
                                 posTcc[:], start=True, stop=True)
                # sq = max(-2G + r2_j + r2_i, 0)
                sq1 = cwork.tile([NP, N], f32, tag="sq1")
                nc.vector.scalar_tensor_tensor(out=sq1[:], in0=g_ps[:],
                                               scalar=-2.0, in1=r2b[:],
                                               op0=ALU.mult, op1=ALU.add)
                sq = cwork.tile([NP, N], f32, tag="sq")
                nc.vector.tensor_scalar(out=sq[:], in0=sq1[:],
                                        scalar1=r2_col[m][:], scalar2=0.0,
                                        op0=ALU.add, op1=ALU.max)
                dist = cwork.tile([NP, N], f32, tag="dist")
                nc.scalar.activation(dist[:], sq[:], AF.Sqrt)
                att0 = cwork.tile([NP, N], f32, tag="att0")
                nc.scalar.activation(att0[:], dist[:], AF.Exp, scale=-1.0 / RADIUS)
                attm = cwork.tile([NP, N], f32, tag="attm")
                nc.vector.scalar_tensor_tensor(out=attm[:], in0=dist[:],
                                               scalar=RADIUS, in1=att0[:],
                                               op0=ALU.is_lt, op1=ALU.mult)
                attz = cwork.tile([NP, N], f32, tag="attz")
                nc.gpsimd.affine_select(out=attz[:], in_=attm[:],
                                        pattern=[[1, N]],
                                        compare_op=ALU.not_equal, fill=0.0,
                                        base=-m * NP, channel_multiplier=-1)
                # feature similarity -> 0.5 + 0.5*cos
                t1 = cwork.tile([NP, N], f32, tag="t1")
                nc.vector.scalar_tensor_tensor(out=t1[:], in0=gf_sb[:],
                                               scalar=rn_col[m][:], in1=rnb[:],
                                               op0=ALU.mult, op1=ALU.mult)
                fs = cwork.tile([NP, N], f32, tag="fs")
                nc.vector.tensor_scalar(out=fs[:], in0=t1[:], scalar1=0.5,
                                        scalar2=0.5, op0=ALU.mult, op1=ALU.add)
                sym = wts.tile([NP, N], f32, tag=f"sym{m}")
                rsc = small.tile([NP, 1], f32, tag=f"rscol{m}")
                nc.vector.scalar_tensor_tensor(out=sym[:], in0=fs[:],
                                               scalar=1.0, in1=attz[:],
                                               op0=ALU.mult, op1=ALU.mult,
                                               accum_out=rsc[:])
                sym_m.append(sym)
                rs_col.append(rsc)

            # per-tile 0.5/(rowsum + 1e-6) columns; row-normalization is applied
            # per output partition in the MP epilogue instead of scaling conn.
            rhalf_m = []
            conn_m = []
            for m in range(NT):
                rsc2 = small.tile([NP, 1], f32, tag=f"rsc2{m}")
                nc.vector.tensor_scalar(out=rsc2[:], in0=rs_col[m][:],
                                        scalar1=1e-6, scalar2=None, op0=ALU.add)
                rrec = small.tile([NP, 1], f32, tag=f"rrec{m}")
                nc.vector.reciprocal(rrec[:], rsc2[:])
                rh = small.tile([NP, 1], f32, tag=f"rhalf{m}")
                nc.vector.tensor_scalar(out=rh[:], in0=rrec[:], scalar1=0.5,
                                        scalar2=None, op0=ALU.mult)
                rhalf_m.append(rh)
                cb = wts.tile([NP, N], bf16, tag=f"conn{m}")
                nc.vector.tensor_copy(cb[:], sym_m[m][:])
                conn_m.append(cb)

            # ---------- phase 1: actT = (x @ iw.T).T * gate + bias ----------
            ps_act = [ps.tile([NP, BS], f32, tag=f"ps{m}", name=f"psact{m}")
                      for m in range(NT)]
            # batched streams: 4 K-tiles per iw DMA, 2 K-tiles per x DMA
            IWB, XB2 = 4, 2
            iw_tiles = {}
            xb_tiles = {}
            for j in range(KT // IWB):
                iw_sb = wts.tile([128, IWB * N], bf16, tag=f"iwg{j}",
                                 name=f"iwg{j}")
                iw_eng = nc.sync if j % 2 == 0 else nc.scalar
                iw_eng.dma_start(
                    out=iw_sb[:].rearrange("p (a n) -> p a n", a=IWB),
                    in_=iwT_d[j * IWB * 128:(j + 1) * IWB * 128, :].rearrange(
                        "(a p) n -> p a n", p=128))
                iw_tiles[j] = iw_sb
            for k in range(KT):
                j, a = k // IWB, k % IWB
                if k % XB2 == 0:
                    g = k // XB2
                    xbt = xbfp.tile([128, XB2 * BS], bf16, tag="xb",
                                    name=f"xbg{g}")
                    x_eng = nc.scalar if g % 2 == 0 else nc.sync
                    x_eng.dma_start(
                        out=xbt[:].rearrange("p (a b) -> p a b", a=XB2),
                        in_=xT_d[g * XB2 * 128:(g + 1) * XB2 * 128, :].rearrange(
                            "(a p) b -> p a b", p=128))
                    xb_tiles[g] = xbt
                xbt = xb_tiles[k // XB2]
                xoff = (k % XB2) * BS
                iw_sb = iw_tiles[j]
                for m in range(NT):
                    for c in range(NCH):
                        nc.tensor.matmul(
                            ps_act[m][:, c * CH:(c + 1) * CH],
                            iw_sb[:, a * N + m * NP:a * N + (m + 1) * NP],
                            xbt[:, xoff + c * CH:xoff + (c + 1) * CH],
                            start=(k == 0), stop=(k == KT - 1))

            act_cur = []
            for m in range(NT):
                a = acts.tile([NP, BS], bf16, tag=f"act{m}")
                nc.vector.tensor_scalar(out=a[:], in0=ps_act[m][:],
                                        scalar1=gate_m[m][:],
                                        scalar2=bias_m[m][:],
                                        op0=ALU.mult, op1=ALU.add)
                act_cur.append(a)

            # ---------- phase 2: message passing ----------
            for it in range(N_ITER):
                ps_mp = [ps.tile([NP, BS], f32, tag=f"ps{m}",
                                 name=f"psmp{it}_{m}") for m in range(NT)]
                for m in range(NT):
                    for a in range(NT):
                        for c in range(NCH):
                            nc.tensor.matmul(
                                ps_mp[m][:, c * CH:(c + 1) * CH],
                                conn_m[a][:, m * NP:(m + 1) * NP],
                                act_cur[a][:, c * CH:(c + 1) * CH],
                                start=(a == 0), stop=(a == NT - 1))
                act_new = []
                for m in range(NT):
                    a2 = acts.tile([NP, BS], bf16, tag=f"act{m}")
                    if it == 0:
                        # first iteration: pre-relu activations can be negative
                        upd = cwork.tile([NP, BS], f32, tag="upd")
                        nc.vector.scalar_tensor_tensor(
                            out=upd[:], in0=ps_mp[m][:], scalar=rhalf_m[m][:],
                            in1=act_cur[m][:], op0=ALU.mult, op1=ALU.add)
                        nc.vector.tensor_scalar(out=a2[:], in0=upd[:],
                                                scalar1=0.0, scalar2=50.0,
                                                op0=ALU.max, op1=ALU.min)
                    else:
                        # act>=0 and conn>=0 => relu/min(50) are no-ops here
                        nc.vector.scalar_tensor_tensor(
                            out=a2[:], in0=ps_mp[m][:], scalar=rhalf_m[m][:],
                            in1=act_cur[m][:], op0=ALU.mult, op1=ALU.add)
                    act_new.append(a2)
                act_cur = act_new

            # ---------- phase 3: output ----------
            ps_y = ps.tile([OUT, BS], f32, tag="ps0")
            for a in range(NT):
                for c in range(NCH):
                    nc.tensor.matmul(ps_y[:, c * CH:(c + 1) * CH],
                                     wtil_m[a][:],
                                     act_cur[a][:, c * CH:(c + 1) * CH],
                                     start=(a == 0), stop=(a == NT - 1))
            y_sb = small.tile([OUT, BS], f32, tag="ysb")
            nc.vector.tensor_copy(y_sb[:], ps_y[:])
            nc.sync.dma_start(out=yT_d[:], in_=y_sb[:])

    nc.compile()
    return nc


def _get_nc():
    if "nc" not in _CACHE:
        _CACHE["nc"] = _build()
    return _CACHE["nc"]


def _run(x, positions, input_weights, features, output_weights, biases,
         trace=False):
    from concourse.bass_utils import run_bass_kernel_spmd
    import concourse.mybir as mybir

    bf16_np = mybir.dt.np(mybir.dt.bfloat16)

    nc = _get_nc()

    x = np.ascontiguousarray(x, dtype=np.float32)
    iwT_bf = np.ascontiguousarray(
        np.asarray(input_weights, dtype=np.float32).T).astype(bf16_np)
    pos = np.ascontiguousarray(positions, dtype=np.float32)
    posT = np.ascontiguousarray(pos.T)
    featT = np.ascontiguousarray(
        np.asarray(features, dtype=np.float32).T)
    ow = np.ascontiguousarray(output_weights, dtype=np.float32)
    bias2 = np.ascontiguousarray(
        np.asarray(biases, dtype=np.float32).reshape(N, 1))

    in_maps = []
    for c in range(NCORES):
        xs = np.ascontiguousarray(x[c * BS:(c + 1) * BS, :].T).astype(bf16_np)
        in_maps.append({
            "xT": xs, "iwT": iwT_bf, "pos": pos, "posT": posT,
            "featT": featT, "ow": ow, "bias": bias2,
        })

    res = run_bass_kernel_spmd(nc, in_maps, list(range(NCORES)), trace=trace)
    y = np.empty((B, OUT), dtype=np.float32)
    for c in range(NCORES):
        y[c * BS:(c + 1) * BS, :] = res.results[c]["yT"].T
    return y, res


def kernel(x, positions, input_weights, features, output_weights, biases):
    y, _ = _run(x, positions, input_weights, features, output_weights, biases)
    return y



# revision 13
# speedup vs baseline: 1.0213x; 1.0213x over previous
"""Trainium2 Bass kernel for GrowingFieldV2 GNN message passing.

Data-parallel over batch: 8 NeuronCores, each processing a 1024-row shard
of x. Small [500,*] parameters are replicated; the [500,500] connectivity
matrix is computed redundantly on every core.

v2 design notes (from trace analysis of v1):
- The PE clock ramps 0.65 -> 1.2 -> 2.4 GHz at fixed wall-clock times
  (~34 us / ~46 us from NEFF start) regardless of activity, so the only
  levers are: start real matmuls early, put cheap (LDW-bound) matmuls in
  the slow window, and minimize total matmul columns.
- MP iterations 2..3 are linear (relu/min never bind after iter 0), so
  they are folded into an effective output weight
  Weff = (M^T)^2 (ow * og), M = I + 0.5*C.  y = act1 @ Weff.
  This removes 64 of 96 [125x512] message-passing matmuls and replaces
  them with 32 tiny [*,10] matmuls scheduled into the slow-clock window.
- Phase 1 runs c-chunked (two passes of [125,512] accumulators) so it
  needs only 4 PSUM banks per pass, freeing the other 4 for the
  connectivity grams / Weff smalls.
- Params are packed into two DRAM tensors (1 DMA each); per-m columns
  live as slices of packed [125,4]-style tiles to cut tile count
  (the NEFF teardown emits per-tile semaphore work).
"""

import sys

for _p in ("/opt/trn_rl_repo",):
    if _p not in sys.path:
        sys.path.insert(0, _p)

import numpy as np

N = 500            # neurons
IN = 3072          # input size
FD = 64            # feature dim
OUT = 10           # output size
B = 8192           # full batch
NCORES = 8
BS = B // NCORES   # 1024 per-core batch shard
RADIUS = 20.0
VOL = 100.0

NT = 4             # neuron tiles
NP = N // NT       # 125 neurons per tile
KT = IN // 128     # 24 contraction k-tiles for phase 1
NCH = 2            # batch chunks of 512 (PSUM bank width)
CH = BS // NCH     # 512
IWG = 8            # k-tiles per iw DMA group
XG = 4             # k-tiles per x DMA group
PCOL = 3 + OUT + 1 + FD   # packed param columns per m-tile (78)

_CACHE = {}


def _build():
    import concourse.bacc as bacc
    import concourse.tile as tile
    import concourse.bass as bass
    import concourse.mybir as mybir

    f32 = mybir.dt.float32
    bf16 = mybir.dt.bfloat16
    AF = mybir.ActivationFunctionType
    ALU = mybir.AluOpType
    PSUM = bass.MemorySpace.PSUM

    nc = bacc.Bacc("TRN2", target_bir_lowering=False, debug=False,
                   num_devices=NCORES)

    xT_d = nc.dram_tensor("xT", [IN, BS], bf16, kind="ExternalInput").ap()
    iwT_d = nc.dram_tensor("iwT", [IN, N], bf16, kind="ExternalInput").ap()
    ptft_d = nc.dram_tensor("ptft", [4 + FD, N], f32, kind="ExternalInput").ap()
    par_d = nc.dram_tensor("par", [NP, NT * PCOL], f32, kind="ExternalInput").ap()
    yT_d = nc.dram_tensor("yT", [OUT, BS], f32, kind="ExternalOutput").ap()

    with tile.TileContext(nc) as tc:
        with (
            tc.tile_pool(name="big", bufs=1) as big,
            tc.tile_pool(name="work", bufs=1) as work,
            tc.tile_pool(name="small", bufs=1) as small,
            tc.tile_pool(name="ps", bufs=1, space=PSUM) as ps,
        ):
            # ---------------- DMAs ----------------
            # scalar queue: ptft first (feeds the early gram matmuls), then iw
            ptft_sb = small.tile([4 + FD, N], f32, tag="ptft")
            nc.scalar.dma_start(out=ptft_sb[:], in_=ptft_d[:])
            iw_sb = []
            for g in range(KT // IWG):
                t = big.tile([128, IWG * N], bf16, tag=f"iwg{g}")
                nc.scalar.dma_start(
                    out=t[:].rearrange("p (a n) -> p a n", a=IWG),
                    in_=iwT_d[g * IWG * 128:(g + 1) * IWG * 128, :].rearrange(
                        "(a p) n -> p a n", p=128))
                iw_sb.append(t)
            # gpsimd queue: packed per-m params
            par_sb = small.tile([NP, NT * PCOL], f32, tag="par")
            nc.gpsimd.dma_start(out=par_sb[:], in_=par_d[:])
            # sync queue: x groups
            x_sb = []
            for g in range(KT // XG):
                t = big.tile([128, XG * BS], bf16, tag=f"xg{g}")
                nc.sync.dma_start(
                    out=t[:].rearrange("p (a b) -> p a b", a=XG),
                    in_=xT_d[g * XG * 128:(g + 1) * XG * 128, :].rearrange(
                        "(a p) b -> p a b", p=128))
                x_sb.append(t)

            def parc(m, lo, hi):
                return par_sb[:, m * PCOL + lo:m * PCOL + hi]

            # ---------------- vector preprocessing ----------------
            posTc = small.tile([3, N], f32, tag="posTc")
            nc.vector.tensor_scalar(out=posTc[:], in0=ptft_sb[FD:FD + 3, :],
                                    scalar1=0.1, scalar2=VOL - 0.1,
                                    op0=ALU.max, op1=ALU.min)
            # A4 = [centered pos; ones], B4 = [-2*centered pos; r2row]
            # (rows at partition 3 are engine-unaddressable; fill via DMA)
            A4 = small.tile([4, N], f32, tag="A4")
            nc.vector.tensor_scalar(out=A4[0:3, :], in0=posTc[:],
                                    scalar1=50.0, scalar2=None,
                                    op0=ALU.subtract)
            nc.gpsimd.dma_start(out=A4[3:4, :], in_=ptft_d[FD + 3:FD + 4, :])
            B4 = small.tile([4, N], f32, tag="B4")
            nc.vector.tensor_scalar(out=B4[0:3, :], in0=A4[0:3, :],
                                    scalar1=-2.0, scalar2=None, op0=ALU.mult)
            pos2 = small.tile([3, N], f32, tag="pos2")
            nc.vector.tensor_mul(pos2[:], A4[0:3, :], A4[0:3, :])
            feat2 = small.tile([FD, N], f32, tag="feat2")
            nc.vector.tensor_mul(feat2[:], ptft_sb[0:FD, :], ptft_sb[0:FD, :])
            ones3 = small.tile([3, 1], f32, tag="ones3")
            nc.vector.memset(ones3[:], 1.0)
            ones64 = small.tile([FD, 1], f32, tag="ones64")
            nc.vector.memset(ones64[:], 1.0)
            ones1 = small.tile([1, NP], f32, tag="ones1")
            nc.vector.memset(ones1[:], 1.0)
            neg2_row = small.tile([1, 1], f32, tag="neg2row")
            nc.vector.memset(neg2_row[:], -2.0)
            neg2_col = small.tile([NP, 1], f32, tag="neg2col")
            nc.vector.memset(neg2_col[:], -2.0)

            # clipped x-coordinate columns, packed [125, 4]
            pox = small.tile([NP, NT], f32, tag="pox")
            for m in range(NT):
                nc.vector.tensor_scalar(out=pox[:, m:m + 1],
                                        in0=parc(m, 0, 1),
                                        scalar1=0.1, scalar2=VOL - 0.1,
                                        op0=ALU.max, op1=ALU.min)
            # per-m squared radius columns r2c [125,4] from packed pos
            pcc = work.tile([NP, 3], f32, tag="pcc")
            pcm = work.tile([NP, 3], f32, tag="pcm")
            pc2 = work.tile([NP, 3], f32, tag="pc2")
            r2c = small.tile([NP, NT], f32, tag="r2c")
            for m in range(NT):
                nc.vector.tensor_scalar(out=pcc[:], in0=parc(m, 0, 3),
                                        scalar1=0.1, scalar2=VOL - 0.1,
                                        op0=ALU.max, op1=ALU.min)
                nc.vector.tensor_scalar(out=pcm[:], in0=pcc[:],
                                        scalar1=50.0, scalar2=None,
                                        op0=ALU.subtract)
                nc.vector.tensor_mul(pc2[:], pcm[:], pcm[:])
                nc.vector.reduce_sum(r2c[:, m:m + 1], pc2[:],
                                     axis=mybir.AxisListType.X)
            # per-m feature norm columns -> rnc [125,4]
            ff = work.tile([NP, FD], f32, tag="ff")
            f2c = small.tile([NP, NT], f32, tag="f2c")
            for m in range(NT):
                nc.vector.tensor_mul(ff[:], parc(m, 14, PCOL), parc(m, 14, PCOL))
                nc.vector.reduce_sum(f2c[:, m:m + 1], ff[:],
                                     axis=mybir.AxisListType.X)
            nrc = small.tile([NP, NT], f32, tag="nrc")
            nc.scalar.activation(nrc[:], f2c[:], AF.Sqrt)
            nrc2 = small.tile([NP, NT], f32, tag="nrc2")
            nc.vector.tensor_scalar(out=nrc2[:], in0=nrc[:], scalar1=1e-6,
                                    scalar2=None, op0=ALU.max)
            rnc = small.tile([NP, NT], f32, tag="rnc")
            nc.vector.reciprocal(rnc[:], nrc2[:])

            # ---------------- early tensor smalls + grams ----------------
            # feature grams (only need ptft): earliest real tensor work
            gfsb = work.tile([NP, NT * N], f32, tag="gfsb")
            for m in range(NT):
                gf_ps = ps.tile([NP, N], f32, tag=f"pb{5 if m % 2 == 0 else 7}",
                                name=f"gf{m}")
                nc.tensor.matmul(gf_ps[:], ptft_sb[0:FD, m * NP:(m + 1) * NP],
                                 ptft_sb[0:FD, :], start=True, stop=True)
                nc.vector.tensor_copy(gfsb[:, m * N:(m + 1) * N], gf_ps[:])

            # r2 row, f2 row, gate sum broadcasts, rn row broadcast
            r2_ps = ps.tile([1, N], f32, tag="pb0", name="r2row")
            nc.tensor.matmul(r2_ps[:], ones3[:], pos2[:], start=True, stop=True)
            r2sb = small.tile([1, N], f32, tag="r2sb")
            nc.vector.tensor_copy(r2sb[:], r2_ps[:])
            nc.gpsimd.dma_start(out=B4[3:4, :], in_=r2sb[:])

            f2_ps = ps.tile([1, N], f32, tag="pb1", name="f2row")
            nc.tensor.matmul(f2_ps[:], ones64[:], feat2[:], start=True, stop=True)
            nrm = small.tile([1, N], f32, tag="nrm")
            nc.scalar.activation(nrm[:], f2_ps[:], AF.Sqrt)
            nrm2 = small.tile([1, N], f32, tag="nrm2")
            nc.vector.tensor_scalar(out=nrm2[:], in0=nrm[:], scalar1=1e-6,
                                    scalar2=None, op0=ALU.max)
            rnrow = small.tile([1, N], f32, tag="rnrow")
            nc.vector.reciprocal(rnrow[:], nrm2[:])

            # input/output gating rows and normalizers
            igrow = small.tile([1, N], f32, tag="igrow")
            nc.scalar.activation(igrow[:], posTc[0:1, :], AF.Exp,
                                 scale=-2.0 / VOL)
            igsum = small.tile([1, 1], f32, tag="igsum")
            nc.vector.reduce_sum(igsum[:], igrow[:], axis=mybir.AxisListType.X)
            ogrow = small.tile([1, N], f32, tag="ogrow")
            nc.scalar.activation(ogrow[:], posTc[0:1, :], AF.Exp,
                                 scale=2.0 / VOL, bias=neg2_row[:])
            ogsum = small.tile([1, 1], f32, tag="ogsum")
            nc.vector.reduce_sum(ogsum[:], ogrow[:], axis=mybir.AxisListType.X)

            igs_ps = ps.tile([NP, 1], f32, tag="pb3", name="igs")
            nc.tensor.matmul(igs_ps[:], ones1[:], igsum[:], start=True, stop=True)
            iga = small.tile([NP, 1], f32, tag="iga")
            nc.vector.tensor_scalar(out=iga[:], in0=igs_ps[:], scalar1=1e-6,
                                    scalar2=None, op0=ALU.add)
            igb = small.tile([NP, 1], f32, tag="igb")
            nc.vector.reciprocal(igb[:], iga[:])
            ogs_ps = ps.tile([NP, 1], f32, tag="pb3", name="ogs")
            nc.tensor.matmul(ogs_ps[:], ones1[:], ogsum[:], start=True, stop=True)
            oga = small.tile([NP, 1], f32, tag="oga")
            nc.vector.tensor_scalar(out=oga[:], in0=ogs_ps[:], scalar1=1e-6,
                                    scalar2=None, op0=ALU.add)
            ogb = small.tile([NP, 1], f32, tag="ogb")
            nc.vector.reciprocal(ogb[:], oga[:])

            # per-m gate columns (packed) + gated output weights
            gate = small.tile([NP, NT], f32, tag="gate")
            ogc = small.tile([NP, NT], f32, tag="ogc")
            wtilg = small.tile([NP, NT * OUT], bf16, tag="wtilg")
            ie = small.tile([NP, NT], f32, tag="ie")
            oe = small.tile([NP, NT], f32, tag="oe")
            for m in range(NT):
                nc.scalar.activation(ie[:, m:m + 1], pox[:, m:m + 1], AF.Exp,
                                     scale=-2.0 / VOL)
                nc.scalar.activation(oe[:, m:m + 1], pox[:, m:m + 1], AF.Exp,
                                     scale=2.0 / VOL, bias=neg2_col[:])
            for m in range(NT):
                nc.vector.tensor_mul(gate[:, m:m + 1], ie[:, m:m + 1], igb[:])
                nc.vector.tensor_mul(ogc[:, m:m + 1], oe[:, m:m + 1], ogb[:])
                nc.vector.tensor_scalar(out=wtilg[:, m * OUT:(m + 1) * OUT],
                                        in0=parc(m, 3, 13),
                                        scalar1=ogc[:, m:m + 1], scalar2=None,
                                        op0=ALU.mult)

            # rn row broadcast [125, N] via ones-matmul, copied to SBUF
            rnb_ps = ps.tile([NP, N], f32, tag="pb2", name="rnb")
            nc.tensor.matmul(rnb_ps[:], ones1[:], rnrow[:], start=True, stop=True)
            rnb_sb = work.tile([NP, N], f32, tag="rnb")
            nc.vector.tensor_copy(rnb_sb[:], rnb_ps[:])

            # position grams -> squared distances -> attenuation -> conn
            conn = []
            rs = small.tile([NP, NT], f32, tag="rs")
            sqt = work.tile([NP, N], f32, tag="sq")
            dist = work.tile([NP, N], f32, tag="dist")
            att0 = work.tile([NP, N], f32, tag="att0")
            attm = work.tile([NP, N], f32, tag="attm")
            attz = work.tile([NP, N], f32, tag="attz")
            t1 = work.tile([NP, N], f32, tag="t1")
            fst = work.tile([NP, N], f32, tag="fs")
            g2_ps = []
            for m in range(NT):
                gp = ps.tile([NP, N], f32, tag=f"pb{4 if m % 2 == 0 else 6}",
                             name=f"g2_{m}")
                nc.tensor.matmul(gp[:], A4[:, m * NP:(m + 1) * NP], B4[:],
                                 start=True, stop=True)
                g2_ps.append(gp)

            for m in range(NT):
                nc.vector.tensor_scalar(out=sqt[:], in0=g2_ps[m][:],
                                        scalar1=r2c[:, m:m + 1], scalar2=0.0,
                                        op0=ALU.add, op1=ALU.max)
                nc.scalar.activation(dist[:], sqt[:], AF.Sqrt)
                nc.scalar.activation(att0[:], dist[:], AF.Exp,
                                     scale=-1.0 / RADIUS)
                nc.vector.scalar_tensor_tensor(out=attm[:], in0=dist[:],
                                               scalar=RADIUS, in1=att0[:],
                                               op0=ALU.is_lt, op1=ALU.mult)
                nc.gpsimd.affine_select(out=attz[:], in_=attm[:],
                                        pattern=[[1, N]],
                                        compare_op=ALU.not_equal, fill=0.0,
                                        base=-m * NP, channel_multiplier=-1)
                nc.vector.scalar_tensor_tensor(out=t1[:],
                                               in0=gfsb[:, m * N:(m + 1) * N],
                                               scalar=rnc[:, m:m + 1],
                                               in1=rnb_sb[:],
                                               op0=ALU.mult, op1=ALU.mult)
                nc.vector.tensor_scalar(out=fst[:], in0=t1[:], scalar1=0.5,
                                        scalar2=0.5, op0=ALU.mult, op1=ALU.add)
                cb = work.tile([NP, N], bf16, tag=f"conn{m}")
                nc.vector.scalar_tensor_tensor(out=cb[:], in0=fst[:],
                                               scalar=1.0, in1=attz[:],
                                               op0=ALU.mult, op1=ALU.mult,
                                               accum_out=rs[:, m:m + 1])
                conn.append(cb)

            # rh = 0.5 / (rowsum + 1e-6), packed [125,4]
            rha = small.tile([NP, NT], f32, tag="rha")
            nc.vector.tensor_scalar(out=rha[:], in0=rs[:], scalar1=1e-6,
                                    scalar2=None, op0=ALU.add)
            rhb = small.tile([NP, NT], f32, tag="rhb")
            nc.vector.reciprocal(rhb[:], rha[:])
            rh = small.tile([NP, NT], f32, tag="rh")
            nc.vector.tensor_scalar(out=rh[:], in0=rhb[:], scalar1=0.5,
                                    scalar2=None, op0=ALU.mult)
            # Vg = rh * (ow * og)  [125, 4*10] bf16
            vg = small.tile([NP, NT * OUT], bf16, tag="vg")
            for m in range(NT):
                nc.vector.tensor_scalar(out=vg[:, m * OUT:(m + 1) * OUT],
                                        in0=wtilg[:, m * OUT:(m + 1) * OUT],
                                        scalar1=rh[:, m:m + 1], scalar2=None,
                                        op0=ALU.mult)

            # ---------------- phase 1 (c-chunked) + Weff smalls ----------
            act0 = big.tile([NP, NT * BS], bf16, tag="act0")

            psW1 = [ps.tile([NP, OUT], f32, tag=f"pb{4 + j}", name=f"s1_{j}")
                    for j in range(NT)]
            psW2 = [ps.tile([NP, OUT], f32, tag=f"pb{4 + j}", name=f"s2_{j}")
                    for j in range(NT)]
            v1 = small.tile([NP, NT * OUT], bf16, tag="v1")
            v1g = small.tile([NP, NT * OUT], bf16, tag="v1g")
            weff = small.tile([NP, NT * OUT], bf16, tag="weff")

            def emit_s1():
                for j in range(NT):
                    for a in range(NT):
                        nc.tensor.matmul(psW1[j][:],
                                         conn[a][:, j * NP:(j + 1) * NP],
                                         vg[:, a * OUT:(a + 1) * OUT],
                                         start=(a == 0), stop=(a == NT - 1))
                for j in range(NT):
                    nc.vector.tensor_add(v1[:, j * OUT:(j + 1) * OUT],
                                         psW1[j][:],
                                         wtilg[:, j * OUT:(j + 1) * OUT])
                    nc.vector.scalar_tensor_tensor(
                        out=v1g[:, j * OUT:(j + 1) * OUT], in0=psW1[j][:],
                        scalar=rh[:, j:j + 1],
                        in1=vg[:, j * OUT:(j + 1) * OUT],
                        op0=ALU.mult, op1=ALU.add)

            def emit_s2():
                for j in range(NT):
                    for a in range(NT):
                        nc.tensor.matmul(psW2[j][:],
                                         conn[a][:, j * NP:(j + 1) * NP],
                                         v1g[:, a * OUT:(a + 1) * OUT],
                                         start=(a == 0), stop=(a == NT - 1))
                for j in range(NT):
                    nc.vector.tensor_add(weff[:, j * OUT:(j + 1) * OUT],
                                         psW2[j][:],
                                         v1[:, j * OUT:(j + 1) * OUT])

            ps_act = {}
            for c in range(NCH):
                for m in range(NT):
                    ps_act[(c, m)] = ps.tile(
                        [NP, CH], f32, tag=f"pb{c * NT + m}",
                        name=f"psact{c}_{m}")
            for c in range(NCH):
                for k in range(KT):
                    g, a = k // IWG, k % IWG
                    gx, ax = k // XG, k % XG
                    for m in range(NT):
                        nc.tensor.matmul(
                            ps_act[(c, m)][:],
                            iw_sb[g][:, a * N + m * NP:a * N + (m + 1) * NP],
                            x_sb[gx][:, ax * BS + c * CH:ax * BS + (c + 1) * CH],
                            start=(k == 0), stop=(k == KT - 1))
                    if c == 0 and k == 1:
                        emit_s1()
                    if c == 0 and k == 3:
                        emit_s2()
                # epilogue: act0 = psum * gate + bias (bf16)
                for m in range(NT):
                    nc.vector.tensor_scalar(
                        out=act0[:, m * BS + c * CH:m * BS + (c + 1) * CH],
                        in0=ps_act[(c, m)][:],
                        scalar1=gate[:, m:m + 1],
                        scalar2=parc(m, 13, 14),
                        op0=ALU.mult, op1=ALU.add)

            # ---------------- message passing iteration 0 ----------------
            act2 = big.tile([NP, NT * BS], bf16, tag="act2")
            upd = work.tile([NP, CH], f32, tag="upd")
            for c in range(NCH):
                for m in range(NT):
                    pmp = ps.tile([NP, CH], f32, tag=f"pb{c * NT + m}",
                                  name=f"psmp{c}_{m}")
                    for a in range(NT):
                        nc.tensor.matmul(
                            pmp[:],
                            conn[a][:, m * NP:(m + 1) * NP],
                            act0[:, a * BS + c * CH:a * BS + (c + 1) * CH],
                            start=(a == 0), stop=(a == NT - 1))
                    nc.vector.scalar_tensor_tensor(
                        out=upd[:], in0=pmp[:], scalar=rh[:, m:m + 1],
                        in1=act0[:, m * BS + c * CH:m * BS + (c + 1) * CH],
                        op0=ALU.mult, op1=ALU.add)
                    nc.vector.tensor_scalar(
                        out=act2[:, m * BS + c * CH:m * BS + (c + 1) * CH],
                        in0=upd[:], scalar1=0.0, scalar2=50.0,
                        op0=ALU.max, op1=ALU.min)

            # ---------------- output ----------------
            y_sb = small.tile([OUT, BS], f32, tag="ysb")
            for c in range(NCH):
                ps_y = ps.tile([OUT, CH], f32, tag=f"pb{0 if c == 0 else 4}",
                               name=f"psy{c}")
                for a in range(NT):
                    nc.tensor.matmul(
                        ps_y[:], weff[:, a * OUT:(a + 1) * OUT],
                        act2[:, a * BS + c * CH:a * BS + (c + 1) * CH],
                        start=(a == 0), stop=(a == NT - 1))
                nc.vector.tensor_copy(y_sb[:, c * CH:(c + 1) * CH], ps_y[:])
                nc.sync.dma_start(out=yT_d[:, c * CH:(c + 1) * CH],
                                  in_=y_sb[:, c * CH:(c + 1) * CH])

    nc.compile()
    return nc


def _get_nc():
    if "nc" not in _CACHE:
        _CACHE["nc"] = _build()
    return _CACHE["nc"]


def _run(x, positions, input_weights, features, output_weights, biases,
         trace=False):
    from concourse.bass_utils import run_bass_kernel_spmd
    import concourse.mybir as mybir

    bf16_np = mybir.dt.np(mybir.dt.bfloat16)

    nc = _get_nc()

    x = np.ascontiguousarray(x, dtype=np.float32)
    iwT_bf = np.ascontiguousarray(
        np.asarray(input_weights, dtype=np.float32).T).astype(bf16_np)
    pos = np.ascontiguousarray(positions, dtype=np.float32)
    feat = np.asarray(features, dtype=np.float32)
    ow = np.asarray(output_weights, dtype=np.float32)
    bias = np.asarray(biases, dtype=np.float32).reshape(N, 1)

    ptft = np.ascontiguousarray(np.concatenate(
        [feat.T, pos.T, np.ones((1, N), np.float32)], axis=0).astype(np.float32))
    par_parts = []
    for m in range(NT):
        sl = slice(m * NP, (m + 1) * NP)
        par_parts.append(np.concatenate(
            [pos[sl], ow[sl], bias[sl], feat[sl]], axis=1))
    par = np.ascontiguousarray(np.concatenate(par_parts, axis=1))

    in_maps = []
    for c in range(NCORES):
        xs = np.ascontiguousarray(x[c * BS:(c + 1) * BS, :].T).astype(bf16_np)
        in_maps.append({
            "xT": xs, "iwT": iwT_bf, "ptft": ptft, "par": par,
        })

    res = run_bass_kernel_spmd(nc, in_maps, list(range(NCORES)), trace=trace)
    y = np.empty((B, OUT), dtype=np.float32)
    for c in range(NCORES):
        y[c * BS:(c + 1) * BS, :] = res.results[c]["yT"].T
    return y, res


def kernel(x, positions, input_weights, features, output_weights, biases):
    y, _ = _run(x, positions, input_weights, features, output_weights, biases)
    return y


# revision 17
# speedup vs baseline: 1.0253x; 1.0039x over previous
"""Trainium2 Bass kernel for GrowingFieldV2 GNN message passing.

Data-parallel over batch: 8 NeuronCores, each processing a 1024-row shard
of x. Small [500,*] parameters are replicated; the [500,500] connectivity
matrix is computed redundantly on every core.

v3 design notes (from trace analysis of v1/v2):
- The PE clock (DVFS) starts at 0.65 GHz and is granted 2.4 GHz only
  after a sustained-activity window; idle gaps trigger a 50%-utilization
  claw-back.  So: start real matmuls as early as possible, keep the
  tensor queue gapless, put cheap/LDW-bound matmuls into the slow
  window, and minimize total matmul columns.
- MP iterations 2..3 are linear (relu/min never bind after iter 0), so
  they fold into an effective output weight Weff = (M^T)^2 (ow*og),
  M = I + 0.5*C; y = act1 @ Weff.  Removes 64 of 96 [125x512] MP
  matmuls; replaced by 32 tiny [*,10] matmuls in the slow window.
- All big gram matmuls run in bf16 (f32 matmuls are 4 cycles/row).
  The pairwise sq distances need consistent rounding: host ships
  bf16(clip(pos)-50) and -2x that; r2 rows/cols are derived from the
  same rounded values on device (f32 reduction, exact cancellation).
- Row->all-partition broadcasts (r2, rn, gate sums) use
  gpsimd.partition_broadcast instead of PE ones-matmuls; row->column
  transposes (r2c, rnc) use tiny SBUF->SBUF DMAs.
- Phase 1 is c-chunked (two passes of [125,512] accumulators): 4 PSUM
  banks per pass, the other 4 free for grams/Weff smalls.
"""

import sys

for _p in ("/opt/trn_rl_repo",):
    if _p not in sys.path:
        sys.path.insert(0, _p)

import numpy as np

N = 500            # neurons
IN = 3072          # input size
FD = 64            # feature dim
OUT = 10           # output size
B = 8192           # full batch
NCORES = 8
BS = B // NCORES   # 1024 per-core batch shard
RADIUS = 20.0
VOL = 100.0

NT = 4             # neuron tiles
NP = N // NT       # 125 neurons per tile
KT = IN // 128     # 24 contraction k-tiles for phase 1
NCH = 2            # batch chunks of 512 (PSUM bank width)
CH = BS // NCH     # 512
IWG = 8            # k-tiles per iw DMA group
XG = 4             # k-tiles per x DMA group
PCOL = 3 + OUT + 1  # packed param columns per m-tile (pos3, ow10, bias1)

_CACHE = {}


def _build():
    import concourse.bacc as bacc
    import concourse.tile as tile
    import concourse.bass as bass
    import concourse.mybir as mybir

    f32 = mybir.dt.float32
    bf16 = mybir.dt.bfloat16
    AF = mybir.ActivationFunctionType
    ALU = mybir.AluOpType
    PSUM = bass.MemorySpace.PSUM

    nc = bacc.Bacc("TRN2", target_bir_lowering=False, debug=False,
                   num_devices=NCORES)

    xT_d = nc.dram_tensor("xT", [IN, BS], bf16, kind="ExternalInput").ap()
    iwT_d = nc.dram_tensor("iwT", [IN, N], bf16, kind="ExternalInput").ap()
    # aux bf16 rows: 0-63 featT, 64-66 bf16(clip(pos).T-50), 67-69 -2x that
    aux_d = nc.dram_tensor("aux", [FD + 6, N], bf16, kind="ExternalInput").ap()
    post_d = nc.dram_tensor("post", [3, N], f32, kind="ExternalInput").ap()
    par_d = nc.dram_tensor("par", [NP, NT * PCOL], f32, kind="ExternalInput").ap()
    scr_d = nc.dram_tensor("rowscr", [2, N], f32, kind="Internal").ap()
    yT_d = nc.dram_tensor("yT", [OUT, BS], f32, kind="ExternalOutput").ap()

    with tile.TileContext(nc) as tc:
        with (
            tc.tile_pool(name="big", bufs=1) as big,
            tc.tile_pool(name="work", bufs=1) as work,
            tc.tile_pool(name="small", bufs=1) as small,
            tc.tile_pool(name="ps", bufs=1, space=PSUM) as ps,
        ):
            # ---------------- DMAs ----------------
            # scalar queue: aux tiles first (feed the early gram matmuls)
            featb = small.tile([FD, N], bf16, tag="featb")
            nc.scalar.dma_start(out=featb[0:32, :], in_=aux_d[0:32, :])
            ccb = small.tile([3, N], bf16, tag="ccb")
            nc.scalar.dma_start(out=ccb[:], in_=aux_d[FD:FD + 3, :])
            ccn2 = small.tile([3, N], bf16, tag="ccn2")
            nc.scalar.dma_start(out=ccn2[:], in_=aux_d[FD + 3:FD + 6, :])
            iw_sb = []
            for g in range(KT // IWG):
                t = big.tile([128, IWG * N], bf16, tag=f"iwg{g}")
                nc.scalar.dma_start(
                    out=t[:].rearrange("p (a n) -> p a n", a=IWG),
                    in_=iwT_d[g * IWG * 128:(g + 1) * IWG * 128, :].rearrange(
                        "(a p) n -> p a n", p=128))
                iw_sb.append(t)
            # gpsimd queue: featb upper half, posT, packed per-m params
            nc.gpsimd.dma_start(out=featb[32:FD, :], in_=aux_d[32:FD, :])
            post_sb = small.tile([3, N], f32, tag="post")
            nc.gpsimd.dma_start(out=post_sb[:], in_=post_d[:])
            par_sb = small.tile([NP, NT * PCOL], f32, tag="par")
            nc.gpsimd.dma_start(out=par_sb[:], in_=par_d[:])
            # sync queue: x groups
            x_sb = []
            for g in range(KT // XG):
                t = big.tile([128, XG * BS], bf16, tag=f"xg{g}")
                nc.sync.dma_start(
                    out=t[:].rearrange("p (a b) -> p a b", a=XG),
                    in_=xT_d[g * XG * 128:(g + 1) * XG * 128, :].rearrange(
                        "(a p) b -> p a b", p=128))
                x_sb.append(t)

            def parc(m, lo, hi):
                return par_sb[:, m * PCOL + lo:m * PCOL + hi]

            # ---------------- vector preprocessing ----------------
            ones3 = small.tile([3, 1], f32, tag="ones3")
            nc.vector.memset(ones3[:], 1.0)
            ones64 = small.tile([FD, 1], bf16, tag="ones64")
            nc.vector.memset(ones64[:], 1.0)
            neg2_row = small.tile([1, 1], f32, tag="neg2row")
            nc.vector.memset(neg2_row[:], -2.0)
            neg2_col = small.tile([NP, 1], f32, tag="neg2col")
            nc.vector.memset(neg2_col[:], -2.0)

            ff2 = small.tile([FD, N], bf16, tag="ff2")
            nc.vector.tensor_mul(ff2[:], featb[:], featb[:])
            pos2b = small.tile([3, N], f32, tag="pos2b")
            nc.vector.tensor_mul(pos2b[:], ccb[:], ccb[:])
            posTc = small.tile([3, N], f32, tag="posTc")
            nc.vector.tensor_scalar(out=posTc[:], in0=post_sb[:],
                                    scalar1=0.1, scalar2=VOL - 0.1,
                                    op0=ALU.max, op1=ALU.min)
            # clipped x-coordinate columns, packed [125, 4]
            pox = small.tile([NP, NT], f32, tag="pox")
            for m in range(NT):
                nc.vector.tensor_scalar(out=pox[:, m:m + 1],
                                        in0=parc(m, 0, 1),
                                        scalar1=0.1, scalar2=VOL - 0.1,
                                        op0=ALU.max, op1=ALU.min)

            # ---------------- early tensor smalls + grams ----------------
            gf_ps = []
            for m in range(2):
                gp = ps.tile([NP, N], f32, tag=f"pb{5 if m % 2 == 0 else 7}",
                             name=f"gf{m}")
                nc.tensor.matmul(gp[:], featb[:, m * NP:(m + 1) * NP],
                                 featb[:], start=True, stop=True)
                gf_ps.append(gp)

            # f2row (bf16) then r2row (f32): rows of squared norms
            f2_ps = ps.tile([1, N], f32, tag="pb1", name="f2row")
            nc.tensor.matmul(f2_ps[:], ones64[:], ff2[:], start=True, stop=True)
            nrm = small.tile([1, N], f32, tag="nrm")
            nc.scalar.activation(nrm[:], f2_ps[:], AF.Sqrt)
            nrm2 = small.tile([1, N], f32, tag="nrm2")
            nc.vector.tensor_scalar(out=nrm2[:], in0=nrm[:], scalar1=1e-6,
                                    scalar2=None, op0=ALU.max)
            rnrow = small.tile([1, N], f32, tag="rnrow")
            nc.vector.reciprocal(rnrow[:], nrm2[:])

            r2_ps = ps.tile([1, N], f32, tag="pb0", name="r2row")
            nc.tensor.matmul(r2_ps[:], ones3[:], pos2b[:], start=True, stop=True)
            r2sb = small.tile([1, N], f32, tag="r2sb")
            nc.vector.tensor_copy(r2sb[:], r2_ps[:])

            # broadcasts via gpsimd; row->col transposes via tiny DMAs
            rnb_sb = work.tile([NP, N], f32, tag="rnb")
            nc.gpsimd.partition_broadcast(rnb_sb[:], rnrow[:])
            r2b_sb = work.tile([NP, N], f32, tag="r2b")
            nc.gpsimd.partition_broadcast(r2b_sb[:], r2sb[:])
            # row -> per-m column transposes via a DRAM round-trip (same
            # gpsimd queue, FIFO-ordered; SBUF->SBUF cross-partition scatter
            # does not lower correctly)
            rnc4 = small.tile([NP, NT], f32, tag="rnc4")
            nc.gpsimd.dma_start(out=scr_d[0:1, :], in_=rnrow[:])
            nc.gpsimd.dma_start(
                out=rnc4[:],
                in_=scr_d[0:1, :].rearrange("a (m p) -> (a p) m", p=NP))
            r2c4 = small.tile([NP, NT], f32, tag="r2c4")
            nc.gpsimd.dma_start(out=scr_d[1:2, :], in_=r2sb[:])
            nc.gpsimd.dma_start(
                out=r2c4[:],
                in_=scr_d[1:2, :].rearrange("a (m p) -> (a p) m", p=NP))

            # input/output gating rows, sums, broadcast normalizers
            igrow = small.tile([1, N], f32, tag="igrow")
            nc.scalar.activation(igrow[:], posTc[0:1, :], AF.Exp,
                                 scale=-2.0 / VOL)
            igsum = small.tile([1, 1], f32, tag="igsum")
            nc.vector.reduce_sum(igsum[:], igrow[:], axis=mybir.AxisListType.X)
            ogrow = small.tile([1, N], f32, tag="ogrow")
            nc.scalar.activation(ogrow[:], posTc[0:1, :], AF.Exp,
                                 scale=2.0 / VOL, bias=neg2_row[:])
            ogsum = small.tile([1, 1], f32, tag="ogsum")
            nc.vector.reduce_sum(ogsum[:], ogrow[:], axis=mybir.AxisListType.X)
            igsb = small.tile([NP, 1], f32, tag="igsb")
            nc.gpsimd.partition_broadcast(igsb[:], igsum[:])
            ogsb = small.tile([NP, 1], f32, tag="ogsb")
            nc.gpsimd.partition_broadcast(ogsb[:], ogsum[:])
            iga = small.tile([NP, 1], f32, tag="iga")
            nc.vector.tensor_scalar(out=iga[:], in0=igsb[:], scalar1=1e-6,
                                    scalar2=None, op0=ALU.add)
            igb = small.tile([NP, 1], f32, tag="igb")
            nc.vector.reciprocal(igb[:], iga[:])
            oga = small.tile([NP, 1], f32, tag="oga")
            nc.vector.tensor_scalar(out=oga[:], in0=ogsb[:], scalar1=1e-6,
                                    scalar2=None, op0=ALU.add)
            ogb = small.tile([NP, 1], f32, tag="ogb")
            nc.vector.reciprocal(ogb[:], oga[:])

            # per-m gate columns (packed) + gated output weights
            gate = small.tile([NP, NT], f32, tag="gate")
            ogc = small.tile([NP, NT], f32, tag="ogc")
            wtilg = small.tile([NP, NT * OUT], bf16, tag="wtilg")
            ie = small.tile([NP, NT], f32, tag="ie")
            oe = small.tile([NP, NT], f32, tag="oe")
            for m in range(NT):
                nc.scalar.activation(ie[:, m:m + 1], pox[:, m:m + 1], AF.Exp,
                                     scale=-2.0 / VOL)
                nc.scalar.activation(oe[:, m:m + 1], pox[:, m:m + 1], AF.Exp,
                                     scale=2.0 / VOL, bias=neg2_col[:])
            for m in range(NT):
                nc.vector.tensor_mul(gate[:, m:m + 1], ie[:, m:m + 1], igb[:])
                nc.vector.tensor_mul(ogc[:, m:m + 1], oe[:, m:m + 1], ogb[:])
                nc.vector.tensor_scalar(out=wtilg[:, m * OUT:(m + 1) * OUT],
                                        in0=parc(m, 3, 13),
                                        scalar1=ogc[:, m:m + 1], scalar2=None,
                                        op0=ALU.mult)

            # position grams (bf16, consistent rounding): tensor queue order
            # gf0 gf1 f2row r2row G2_0 G2_1 gf2 gf3 G2_2 G2_3
            g2_ps = []
            for m in range(2):
                gp = ps.tile([NP, N], f32, tag=f"pb{4 if m % 2 == 0 else 6}",
                             name=f"g2_{m}")
                nc.tensor.matmul(gp[:], ccb[:, m * NP:(m + 1) * NP], ccn2[:],
                                 start=True, stop=True)
                g2_ps.append(gp)
            for m in range(2, NT):
                gp = ps.tile([NP, N], f32, tag=f"pb{5 if m % 2 == 0 else 7}",
                             name=f"gf{m}")
                nc.tensor.matmul(gp[:], featb[:, m * NP:(m + 1) * NP],
                                 featb[:], start=True, stop=True)
                gf_ps.append(gp)
            for m in range(2, NT):
                gp = ps.tile([NP, N], f32, tag=f"pb{4 if m % 2 == 0 else 6}",
                             name=f"g2_{m}")
                nc.tensor.matmul(gp[:], ccb[:, m * NP:(m + 1) * NP], ccn2[:],
                                 start=True, stop=True)
                g2_ps.append(gp)

            # conn chain: sq -> dist -> att -> mask -> * feat-sim -> conn
            conn = []
            rs = small.tile([NP, NT], f32, tag="rs")
            sqt = work.tile([NP, N], f32, tag="sq")
            sqm = work.tile([NP, N], f32, tag="sqm")
            dist = work.tile([NP, N], f32, tag="dist")
            att0 = work.tile([NP, N], f32, tag="att0")
            attm = work.tile([NP, N], f32, tag="attm")
            attz = work.tile([NP, N], f32, tag="attz")
            t1 = work.tile([NP, N], f32, tag="t1")
            fst = work.tile([NP, N], f32, tag="fs")
            for m in range(NT):
                nc.vector.scalar_tensor_tensor(out=sqt[:], in0=g2_ps[m][:],
                                               scalar=r2c4[:, m:m + 1],
                                               in1=r2b_sb[:],
                                               op0=ALU.add, op1=ALU.add)
                nc.vector.tensor_scalar(out=sqm[:], in0=sqt[:], scalar1=0.0,
                                        scalar2=None, op0=ALU.max)
                nc.scalar.activation(dist[:], sqm[:], AF.Sqrt)
                nc.scalar.activation(att0[:], dist[:], AF.Exp,
                                     scale=-1.0 / RADIUS)
                nc.vector.scalar_tensor_tensor(out=attm[:], in0=dist[:],
                                               scalar=RADIUS, in1=att0[:],
                                               op0=ALU.is_lt, op1=ALU.mult)
                nc.gpsimd.affine_select(out=attz[:], in_=attm[:],
                                        pattern=[[1, N]],
                                        compare_op=ALU.not_equal, fill=0.0,
                                        base=-m * NP, channel_multiplier=-1)
                nc.vector.scalar_tensor_tensor(out=t1[:], in0=gf_ps[m][:],
                                               scalar=rnc4[:, m:m + 1],
                                               in1=rnb_sb[:],
                                               op0=ALU.mult, op1=ALU.mult)
                nc.vector.tensor_scalar(out=fst[:], in0=t1[:], scalar1=0.5,
                                        scalar2=0.5, op0=ALU.mult, op1=ALU.add)
                cb = work.tile([NP, N], bf16, tag=f"conn{m}")
                nc.vector.scalar_tensor_tensor(out=cb[:], in0=fst[:],
                                               scalar=1.0, in1=attz[:],
                                               op0=ALU.mult, op1=ALU.mult,
                                               accum_out=rs[:, m:m + 1])
                conn.append(cb)

            # rh = 0.5 / (rowsum + 1e-6), packed [125,4]
            rha = small.tile([NP, NT], f32, tag="rha")
            nc.vector.tensor_scalar(out=rha[:], in0=rs[:], scalar1=1e-6,
                                    scalar2=None, op0=ALU.add)
            rhb = small.tile([NP, NT], f32, tag="rhb")
            nc.vector.reciprocal(rhb[:], rha[:])
            rh = small.tile([NP, NT], f32, tag="rh")
            nc.vector.tensor_scalar(out=rh[:], in0=rhb[:], scalar1=0.5,
                                    scalar2=None, op0=ALU.mult)
            # Vg = rh * (ow * og)  [125, 4*10] bf16
            vg = small.tile([NP, NT * OUT], bf16, tag="vg")
            for m in range(NT):
                nc.vector.tensor_scalar(out=vg[:, m * OUT:(m + 1) * OUT],
                                        in0=wtilg[:, m * OUT:(m + 1) * OUT],
                                        scalar1=rh[:, m:m + 1], scalar2=None,
                                        op0=ALU.mult)

            # ---------------- phase 1 (c-chunked) + Weff smalls ----------
            act0 = big.tile([NP, NT * BS], bf16, tag="act0")

            psW1 = [ps.tile([NP, OUT], f32, tag=f"pb{4 + j}", name=f"s1_{j}")
                    for j in range(NT)]
            psW2 = [ps.tile([NP, OUT], f32, tag=f"pb{4 + j}", name=f"s2_{j}")
                    for j in range(NT)]
            v1 = small.tile([NP, NT * OUT], bf16, tag="v1")
            v1g = small.tile([NP, NT * OUT], bf16, tag="v1g")
            weff = small.tile([NP, NT * OUT], bf16, tag="weff")

            def emit_s1():
                for j in range(NT):
                    for a in range(NT):
                        nc.tensor.matmul(psW1[j][:],
                                         conn[a][:, j * NP:(j + 1) * NP],
                                         vg[:, a * OUT:(a + 1) * OUT],
                                         start=(a == 0), stop=(a == NT - 1))
                for j in range(NT):
                    nc.vector.tensor_add(v1[:, j * OUT:(j + 1) * OUT],
                                         psW1[j][:],
                                         wtilg[:, j * OUT:(j + 1) * OUT])
                    nc.vector.scalar_tensor_tensor(
                        out=v1g[:, j * OUT:(j + 1) * OUT], in0=psW1[j][:],
                        scalar=rh[:, j:j + 1],
                        in1=vg[:, j * OUT:(j + 1) * OUT],
                        op0=ALU.mult, op1=ALU.add)

            def emit_s2():
                for j in range(NT):
                    for a in range(NT):
                        nc.tensor.matmul(psW2[j][:],
                                         conn[a][:, j * NP:(j + 1) * NP],
                                         v1g[:, a * OUT:(a + 1) * OUT],
                                         start=(a == 0), stop=(a == NT - 1))
                for j in range(NT):
                    nc.vector.tensor_add(weff[:, j * OUT:(j + 1) * OUT],
                                         psW2[j][:],
                                         v1[:, j * OUT:(j + 1) * OUT])

            ps_act = {}
            for c in range(NCH):
                for m in range(NT):
                    ps_act[(c, m)] = ps.tile(
                        [NP, CH], f32, tag=f"pb{c * NT + m}",
                        name=f"psact{c}_{m}")
            for c in range(NCH):
                for k in range(KT):
                    g, a = k // IWG, k % IWG
                    gx, ax = k // XG, k % XG
                    for m in range(NT):
                        nc.tensor.matmul(
                            ps_act[(c, m)][:],
                            iw_sb[g][:, a * N + m * NP:a * N + (m + 1) * NP],
                            x_sb[gx][:, ax * BS + c * CH:ax * BS + (c + 1) * CH],
                            start=(k == 0), stop=(k == KT - 1))
                    if c == 0 and k == 2:
                        emit_s1()
                    if c == 0 and k == 5:
                        emit_s2()
                # epilogue: act0 = psum * gate + bias (bf16)
                for m in range(NT):
                    nc.vector.tensor_scalar(
                        out=act0[:, m * BS + c * CH:m * BS + (c + 1) * CH],
                        in0=ps_act[(c, m)][:],
                        scalar1=gate[:, m:m + 1],
                        scalar2=parc(m, 13, 14),
                        op0=ALU.mult, op1=ALU.add)

            # ---------------- message passing iteration 0 ----------------
            act2 = big.tile([NP, NT * BS], bf16, tag="act2")
            upd = work.tile([NP, CH], f32, tag="upd")
            for c in range(NCH):
                for m in range(NT):
                    pmp = ps.tile([NP, CH], f32, tag=f"pb{c * NT + m}",
                                  name=f"psmp{c}_{m}")
                    for a in range(NT):
                        nc.tensor.matmul(
                            pmp[:],
                            conn[a][:, m * NP:(m + 1) * NP],
                            act0[:, a * BS + c * CH:a * BS + (c + 1) * CH],
                            start=(a == 0), stop=(a == NT - 1))
                    nc.vector.scalar_tensor_tensor(
                        out=upd[:], in0=pmp[:], scalar=rh[:, m:m + 1],
                        in1=act0[:, m * BS + c * CH:m * BS + (c + 1) * CH],
                        op0=ALU.mult, op1=ALU.add)
                    nc.vector.tensor_scalar(
                        out=act2[:, m * BS + c * CH:m * BS + (c + 1) * CH],
                        in0=upd[:], scalar1=0.0, scalar2=50.0,
                        op0=ALU.max, op1=ALU.min)

            # ---------------- output ----------------
            y_sb = small.tile([OUT, BS], f32, tag="ysb")
            for c in range(NCH):
                ps_y = ps.tile([OUT, CH], f32, tag=f"pb{0 if c == 0 else 4}",
                               name=f"psy{c}")
                for a in range(NT):
                    nc.tensor.matmul(
                        ps_y[:], weff[:, a * OUT:(a + 1) * OUT],
                        act2[:, a * BS + c * CH:a * BS + (c + 1) * CH],
                        start=(a == 0), stop=(a == NT - 1))
                nc.vector.tensor_copy(y_sb[:, c * CH:(c + 1) * CH], ps_y[:])
                nc.sync.dma_start(out=yT_d[:, c * CH:(c + 1) * CH],
                                  in_=y_sb[:, c * CH:(c + 1) * CH])

    nc.compile()
    return nc


def _get_nc():
    if "nc" not in _CACHE:
        _CACHE["nc"] = _build()
    return _CACHE["nc"]


def _run(x, positions, input_weights, features, output_weights, biases,
         trace=False):
    from concourse.bass_utils import run_bass_kernel_spmd
    import concourse.mybir as mybir

    bf16_np = mybir.dt.np(mybir.dt.bfloat16)

    nc = _get_nc()

    x = np.ascontiguousarray(x, dtype=np.float32)
    iwT_bf = np.ascontiguousarray(
        np.asarray(input_weights, dtype=np.float32).T).astype(bf16_np)
    pos = np.ascontiguousarray(positions, dtype=np.float32)
    feat = np.asarray(features, dtype=np.float32)
    ow = np.asarray(output_weights, dtype=np.float32)
    bias = np.asarray(biases, dtype=np.float32).reshape(N, 1)

    ccT = (np.clip(pos, 0.1, VOL - 0.1) - 50.0).T.astype(bf16_np)
    aux = np.ascontiguousarray(np.concatenate(
        [feat.T.astype(bf16_np), ccT, (-2.0 * ccT.astype(np.float32)
                                       ).astype(bf16_np)], axis=0))
    post = np.ascontiguousarray(pos.T)
    par_parts = []
    for m in range(NT):
        sl = slice(m * NP, (m + 1) * NP)
        par_parts.append(np.concatenate(
            [pos[sl], ow[sl], bias[sl]], axis=1))
    par = np.ascontiguousarray(np.concatenate(par_parts, axis=1))

    in_maps = []
    for c in range(NCORES):
        xs = np.ascontiguousarray(x[c * BS:(c + 1) * BS, :].T).astype(bf16_np)
        in_maps.append({
            "xT": xs, "iwT": iwT_bf, "aux": aux, "post": post, "par": par,
        })

    res = run_bass_kernel_spmd(nc, in_maps, list(range(NCORES)), trace=trace)
    y = np.empty((B, OUT), dtype=np.float32)
    for c in range(NCORES):
        y[c * BS:(c + 1) * BS, :] = res.results[c]["yT"].T
    return y, res


def kernel(x, positions, input_weights, features, output_weights, biases):
    y, _ = _run(x, positions, input_weights, features, output_weights, biases)
    return y
